# revision 9
# baseline (speedup 1.0000x reference)
"""Trainium2 Bass kernel for KMeans assignment (argmin over centroid distances).

Problem: x [131072, 768] f32, centroids [768, 2000] f32
Output:  argmin_k ||x_n - c_k||^2  -> int32 [131072]

Math: argmin_k(||x||^2 - 2 x.c_k + ||c_k||^2) = argmax_k(x.c_k - 0.5||c_k||^2).
Per-core (data-parallel over 8 cores, 16384 rows each):
  - keep centroids resident in SBUF (bf16 hi/lo split for near-fp32 matmul
    precision at bf16 PE rate: x.c = xh.ch + xh.cl + xl.ch, dropping xl.cl)
  - per 128-row tile: DMA x, cast hi/lo, PE-transpose to [d, n] weights,
    matmul-accumulate scores into PSUM, DVE adds bias (-0.5||c||^2, broadcast
    from host), DVE max/max_index gives argmax along free axis.
"""

import os
import sys

for _p in ("/opt/trn_rl_repo",):
    if _p not in sys.path and os.path.isdir(_p):
        sys.path.insert(0, _p)

from contextlib import ExitStack

import numpy as np

import concourse.bass as bass
import concourse.tile as tile
from concourse import bacc, mybir
from concourse.bass_utils import run_bass_kernel_spmd

try:
    import ml_dtypes

    BF16 = np.dtype(ml_dtypes.bfloat16)
except ImportError:  # pragma: no cover
    BF16 = None

N, D, K = 131072, 768, 2000
NCORES = 8
NSH = N // NCORES  # 16384 rows per core
P = 128
DT = D // P  # 6 contraction tiles
# score chunks, each within one PSUM bank (<=512 fp32)
KOFF = [0, 512, 1024, 1536]
KW = [512, 512, 512, 464]
NB = 4

F32 = mybir.dt.float32
BF = mybir.dt.bfloat16
U32 = mybir.dt.uint32


def build_nc_screen(n_rows: int = NSH):
    """Phase-1 screening program: single bf16 matmul pass.

    Bias (-0.5||c||^2) is folded into the matmul as two extra contraction
    rows (ones-weights x [bias_hi; bias_lo]) so the vector engine only runs
    max/max_index. Outputs the argmax index and the top-2 score values per
    row; rows with a small top-2 margin get recomputed exactly in phase 2.
    """
    assert n_rows % P == 0
    nt = n_rows // P
    nc = bacc.Bacc("TRN2", target_bir_lowering=False, debug=False)

    x = nc.dram_tensor("x", [n_rows, D], F32, kind="ExternalInput").ap()
    c_in = nc.dram_tensor("c", [D, K], BF, kind="ExternalInput").ap()
    bias2 = nc.dram_tensor("bias2", [2, K], BF, kind="ExternalInput").ap()
    ones = nc.dram_tensor("ones", [2, P], BF, kind="ExternalInput").ap()
    out = nc.dram_tensor("out", [n_rows, 1], U32, kind="ExternalOutput").ap()
    vals = nc.dram_tensor("vals", [n_rows, 2], F32, kind="ExternalOutput").ap()

    with tile.TileContext(nc) as tc, ExitStack() as ctx:
        const = ctx.enter_context(tc.tile_pool(name="const", bufs=1))
        xin_p = ctx.enter_context(tc.tile_pool(name="xin", bufs=3))
        xcast_p = ctx.enter_context(tc.tile_pool(name="xcast", bufs=2))
        xts_p = ctx.enter_context(tc.tile_pool(name="xts", bufs=2))
        sc_p = ctx.enter_context(tc.tile_pool(name="sc", bufs=2, space="PSUM"))
        ss_p = ctx.enter_context(tc.tile_pool(name="ss", bufs=2))
        mx_p = ctx.enter_context(tc.tile_pool(name="mx", bufs=4))

        c3 = c_in.rearrange("(t p) k -> t p k", p=P)
        c_tiles = []
        for d in range(DT):
            ct = const.tile([P, K], BF, tag=f"c_{d}")
            nc.sync.dma_start(ct[:], c3[d])
            c_tiles.append(ct)
        bias_t = const.tile([2, K], BF, tag="bias2")
        nc.sync.dma_start(bias_t[:], bias2[:, :])
        ones_t = const.tile([2, P], BF, tag="ones")
        nc.sync.dma_start(ones_t[:], ones[:, :])

        for t in range(nt):
            xin = xin_p.tile([P, D], F32)
            nc.sync.dma_start(xin[:], x[t * P:(t + 1) * P, :])
            xh = xcast_p.tile([P, D], BF, tag="xh")
            nc.scalar.copy(xh[:], xin[:])
            # transpose via DMA xbar (2-byte dtype): keeps PE free and PSUM
            # banks available for double-buffered score accumulation
            xts = xts_p.tile([P, D], BF)
            for d in range(DT):
                nc.sync.dma_start(
                    xts[:, d * P:(d + 1) * P], xh[:, d * P:(d + 1) * P],
                    transpose=True)

            banks = []
            for b in range(NB):
                bank_tile = sc_p.tile([P, KW[b]], F32, tag=f"b{b}", name=f"bank{b}")
                banks.append(bank_tile)
            # bias rows first so each bank's accumulation closes on d == DT-1
            for b in range(NB):
                nc.tensor.matmul(
                    banks[b][:], ones_t[:],
                    bias_t[:, KOFF[b]:KOFF[b] + KW[b]],
                    start=True, stop=False)
            for d in range(DT):
                for b in range(NB):
                    nc.tensor.matmul(
                        banks[b][:], xts[:, d * P:(d + 1) * P],
                        c_tiles[d][:, KOFF[b]:KOFF[b] + KW[b]],
                        start=False, stop=(d == DT - 1))

            ss = ss_p.tile([P, K], F32)
            for b in range(NB):
                nc.scalar.copy(ss[:, KOFF[b]:KOFF[b] + KW[b]], banks[b][:])

            mxv = mx_p.tile([P, 8], F32, tag="mxv")
            nc.vector.max(mxv[:], ss[:])
            mxi = mx_p.tile([P, 8], U32, tag="mxi")
            nc.vector.max_index(mxi[:], mxv[:], ss[:])
            nc.sync.dma_start(out[t * P:(t + 1) * P, :], mxi[:, 0:1])
            nc.sync.dma_start(vals[t * P:(t + 1) * P, :], mxv[:, 0:2])

    nc.compile()
    return nc


def build_nc(mode: str = "bf16x3", n_rows: int = NSH):
    """Build + compile the per-core Bass program.

    mode: 'bf16x3' (hi/lo split, 3 bf16 passes), 'fp32', 'fp32r', 'bf16'
    """
    assert n_rows % P == 0
    nt = n_rows // P
    nc = bacc.Bacc("TRN2", target_bir_lowering=False, debug=False)

    x = nc.dram_tensor("x", [n_rows, D], F32, kind="ExternalInput").ap()
    bias = nc.dram_tensor("bias", [P, K], F32, kind="ExternalInput").ap()
    out = nc.dram_tensor("out", [n_rows, 1], U32, kind="ExternalOutput").ap()

    split = mode == "bf16x3"
    cdt = BF if mode in ("bf16x3", "bf16") else F32
    mmdt = {"bf16x3": BF, "bf16": BF, "fp32": F32, "fp32r": mybir.dt.float32r}[mode]

    if split:
        c_hi = nc.dram_tensor("c_hi", [D, K], BF, kind="ExternalInput").ap()
        c_lo = nc.dram_tensor("c_lo", [D, K], BF, kind="ExternalInput").ap()
        c_srcs = [c_hi, c_lo]
    else:
        c_full = nc.dram_tensor("c", [D, K], cdt, kind="ExternalInput").ap()
        c_srcs = [c_full]
    ident = nc.dram_tensor("ident", [P, P], mmdt if mmdt != mybir.dt.float32r else F32,
                           kind="ExternalInput").ap()

    with tile.TileContext(nc) as tc, ExitStack() as ctx:
        const = ctx.enter_context(tc.tile_pool(name="const", bufs=1))
        xin_p = ctx.enter_context(tc.tile_pool(name="xin", bufs=3))
        xcast_p = ctx.enter_context(tc.tile_pool(name="xcast", bufs=2))
        xtp_p = ctx.enter_context(tc.tile_pool(name="xtp", bufs=2, space="PSUM"))
        xts_p = ctx.enter_context(tc.tile_pool(name="xts", bufs=2))
        sc_p = ctx.enter_context(tc.tile_pool(name="sc", bufs=1, space="PSUM"))
        ss_p = ctx.enter_context(tc.tile_pool(name="ss", bufs=2))
        mx_p = ctx.enter_context(tc.tile_pool(name="mx", bufs=4))

        # centroids resident in SBUF: [DT][P, K] per source (hi/lo or single)
        c_tiles = []
        for si, csrc in enumerate(c_srcs):
            c3 = csrc.rearrange("(t p) k -> t p k", p=P)
            tiles = []
            for d in range(DT):
                ct = const.tile([P, K], cdt, tag=f"c{si}_{d}")
                nc.sync.dma_start(ct[:], c3[d])
                tiles.append(ct)
            c_tiles.append(tiles)

        bias_t = const.tile([P, K], F32, tag="bias")
        nc.sync.dma_start(bias_t[:], bias[:, :])
        id_t = const.tile([P, P], ident.dtype, tag="ident")
        nc.sync.dma_start(id_t[:], ident[:, :])

        for t in range(nt):
            xin = xin_p.tile([P, D], F32)
            nc.sync.dma_start(xin[:], x[t * P:(t + 1) * P, :])

            if split:
                xh = xcast_p.tile([P, D], BF, tag="xh")
                nc.scalar.copy(xh[:], xin[:])
                xl = xcast_p.tile([P, D], BF, tag="xl")
                nc.vector.tensor_sub(xl[:], xin[:], xh[:])
                tsrc = [xh, xl]
            elif mode == "bf16":
                xh = xcast_p.tile([P, D], BF, tag="xh")
                nc.scalar.copy(xh[:], xin[:])
                tsrc = [xh]
            else:
                tsrc = [xin]

            # transpose x tiles -> [d, n] layout for matmul weights
            nsrc = len(tsrc)
            tdt = BF if cdt == BF else F32
            xtp = xtp_p.tile([P, D * nsrc], tdt)
            for si, xsrc in enumerate(tsrc):
                for d in range(DT):
                    nc.tensor.transpose(
                        xtp[:, si * D + d * P: si * D + (d + 1) * P],
                        xsrc[:, d * P:(d + 1) * P],
                        id_t[:],
                    )
            xts = xts_p.tile([P, D * nsrc], tdt)
            nc.scalar.copy(xts[:], xtp[:])

            def w(si, d):
                return xts[:, si * D + d * P: si * D + (d + 1) * P]

            banks = []
            for b in range(NB):
                bank_tile = sc_p.tile([P, KW[b]], F32, tag=f"b{b}", name=f"bank{b}")
                banks.append(bank_tile)
            if split:
                # accumulate xh.ch + xh.cl + xl.ch over d
                for d in range(DT):
                    for b in range(NB):
                        nc.tensor.matmul(
                            banks[b][:], w(0, d),
                            c_tiles[0][d][:, KOFF[b]:KOFF[b] + KW[b]],
                            start=(d == 0), stop=False)
                    for b in range(NB):
                        nc.tensor.matmul(
                            banks[b][:], w(0, d),
                            c_tiles[1][d][:, KOFF[b]:KOFF[b] + KW[b]],
                            start=False, stop=False)
                    for b in range(NB):
                        nc.tensor.matmul(
                            banks[b][:], w(1, d),
                            c_tiles[0][d][:, KOFF[b]:KOFF[b] + KW[b]],
                            start=False, stop=(d == DT - 1))
            else:
                for d in range(DT):
                    for b in range(NB):
                        lhs = w(0, d)
                        rhs = c_tiles[0][d][:, KOFF[b]:KOFF[b] + KW[b]]
                        if mode == "fp32r":
                            lhs = lhs.bitcast(mybir.dt.float32r)
                            rhs = rhs.bitcast(mybir.dt.float32r)
                        nc.tensor.matmul(banks[b][:], lhs, rhs,
                                         start=(d == 0), stop=(d == DT - 1))

            ss = ss_p.tile([P, K], F32)
            for b in range(NB):
                nc.vector.tensor_add(
                    ss[:, KOFF[b]:KOFF[b] + KW[b]], banks[b][:],
                    bias_t[:, KOFF[b]:KOFF[b] + KW[b]])

            mxv = mx_p.tile([P, 8], F32, tag="mxv")
            nc.vector.max(mxv[:], ss[:])
            mxi = mx_p.tile([P, 8], U32, tag="mxi")
            nc.vector.max_index(mxi[:], mxv[:], ss[:])
            nc.sync.dma_start(out[t * P:(t + 1) * P, :], mxi[:, 0:1])

    nc.compile()
    return nc


def make_in_maps(x: np.ndarray, centroids: np.ndarray, mode: str = "bf16x3",
                 n_rows: int = NSH, n_cores: int = NCORES):
    x = np.ascontiguousarray(x, dtype=np.float32)
    c = np.ascontiguousarray(centroids, dtype=np.float32)
    c_norm = (c.astype(np.float64) ** 2).sum(axis=0)
    bias = np.broadcast_to((-0.5 * c_norm).astype(np.float32), (P, K)).copy()

    base = {"bias": bias}
    if mode == "bf16x3":
        c_hi = c.astype(BF16)
        c_lo = (c - c_hi.astype(np.float32)).astype(BF16)
        base["c_hi"] = c_hi
        base["c_lo"] = c_lo
        base["ident"] = np.eye(P, dtype=BF16)
    elif mode == "bf16":
        base["c"] = c.astype(BF16)
        base["ident"] = np.eye(P, dtype=BF16)
    else:
        base["c"] = c
        base["ident"] = np.eye(P, dtype=np.float32)

    in_maps = []
    for i in range(n_cores):
        m = dict(base)
        m["x"] = x[i * n_rows:(i + 1) * n_rows]
        in_maps.append(m)
    return in_maps


_NC_CACHE = {}
LAST_RESULTS = []  # (label, BassKernelResults) of the most recent kernel() call


def _run_spmd(nc, in_maps, label):
    kw = {}
    if os.environ.get("KMEANS_TRACE"):
        kw["trace"] = True
        kw["tmpdir"] = os.environ.get("KMEANS_TRACE_DIR", "/tmp/km_trace") + "_" + label
        os.makedirs(kw["tmpdir"], exist_ok=True)
    res = run_bass_kernel_spmd(nc, in_maps, core_ids=list(range(NCORES)), **kw)
    LAST_RESULTS.append((label, res))
    return res

# Phase-2 capacity: rows per core recomputed exactly. Margin threshold:
# empirical max bf16 score error on randn data is ~0.2; flag anything under
# 4x that. ~5% of rows get flagged at this threshold.
P2_ROWS = 1024
MARGIN_TH = None  # set below after calibration constant


def _cached_nc(key, builder):
    if key not in _NC_CACHE:
        _NC_CACHE[key] = builder()
    return _NC_CACHE[key]


def make_screen_in_maps(x: np.ndarray, centroids: np.ndarray,
                        n_rows: int = NSH, n_cores: int = NCORES):
    x = np.ascontiguousarray(x, dtype=np.float32)
    c = np.ascontiguousarray(centroids, dtype=np.float32)
    c_norm = (c.astype(np.float64) ** 2).sum(axis=0)
    bias = (-0.5 * c_norm).astype(np.float32)
    bias_hi = bias.astype(BF16)
    bias_lo = (bias - bias_hi.astype(np.float32)).astype(BF16)
    base = {
        "c": c.astype(BF16),
        "bias2": np.stack([bias_hi, bias_lo]),
        "ones": np.ones((2, P), dtype=BF16),
    }
    in_maps = []
    for i in range(n_cores):
        m = dict(base)
        m["x"] = x[i * n_rows:(i + 1) * n_rows]
        in_maps.append(m)
    return in_maps


def _run_exact(x_rows: np.ndarray, centroids: np.ndarray, n_rows: int):
    """Run the exact (bf16x3) program on x_rows padded to n_rows*NCORES."""
    nc = _cached_nc(("bf16x3", n_rows), lambda: build_nc("bf16x3", n_rows))
    total = n_rows * NCORES
    xp = np.zeros((total, D), dtype=np.float32)
    xp[: len(x_rows)] = x_rows
    in_maps = make_in_maps(xp, centroids, mode="bf16x3", n_rows=n_rows)
    res = _run_spmd(nc, in_maps, "phase2")
    out = np.concatenate(
        [res.results[i]["out"].reshape(n_rows) for i in range(NCORES)])
    return out[: len(x_rows)], res


def kernel(x: np.ndarray, centroids: np.ndarray) -> np.ndarray:
    mode = os.environ.get("KMEANS_MODE", "hybrid")
    LAST_RESULTS.clear()
    x = np.asarray(x)
    centroids = np.asarray(centroids)

    if mode != "hybrid":
        nc = _cached_nc((mode, NSH), lambda: build_nc(mode=mode))
        in_maps = make_in_maps(x, centroids, mode=mode)
        res = _run_spmd(nc, in_maps, mode)
        parts = [res.results[i]["out"].reshape(NSH) for i in range(NCORES)]
        return np.concatenate(parts).astype(np.int32)

    # phase 1: bf16 screen with top-2 margins
    nc1 = _cached_nc(("screen", NSH), lambda: build_nc_screen(NSH))
    in_maps = make_screen_in_maps(x, centroids)
    res1 = _run_spmd(nc1, in_maps, "phase1")
    idx = np.concatenate(
        [res1.results[i]["out"].reshape(NSH) for i in range(NCORES)]
    ).astype(np.int32)
    vals = np.concatenate(
        [res1.results[i]["vals"].reshape(NSH, 2) for i in range(NCORES)])

    margin = vals[:, 0] - vals[:, 1]
    th = float(os.environ.get("KMEANS_MARGIN_TH", "0.8"))
    flagged = np.flatnonzero(margin < th)

    # phase 2: exact recompute of flagged rows; pick the smallest padded
    # program that covers the count, chunking in the (unexpected) overflow case
    sizes = [512, 1024, 2048]
    per_core = min((s for s in sizes if s * NCORES >= len(flagged)),
                   default=sizes[-1])
    cap = per_core * NCORES
    for s in range(0, len(flagged), cap):
        rows = flagged[s:s + cap]
        exact_idx, _ = _run_exact(x[rows], centroids, per_core)
        idx[rows] = exact_idx
    return idx


# revision 12
# speedup vs baseline: 2.9714x; 2.9714x over previous
"""Trainium2 Bass kernel for KMeans assignment (argmin over centroid distances).

Problem: x [131072, 768] f32, centroids [768, 2000] f32
Output:  argmin_k ||x_n - c_k||^2  -> int32 [131072]

Math: argmin_k(||x||^2 - 2 x.c_k + ||c_k||^2) = argmax_k(x.c_k - 0.5||c_k||^2).
Per-core (data-parallel over 8 cores, 16384 rows each):
  - keep centroids resident in SBUF (bf16 hi/lo split for near-fp32 matmul
    precision at bf16 PE rate: x.c = xh.ch + xh.cl + xl.ch, dropping xl.cl)
  - per 128-row tile: DMA x, cast hi/lo, PE-transpose to [d, n] weights,
    matmul-accumulate scores into PSUM, DVE adds bias (-0.5||c||^2, broadcast
    from host), DVE max/max_index gives argmax along free axis.
"""

import os
import sys

for _p in ("/opt/trn_rl_repo",):
    if _p not in sys.path and os.path.isdir(_p):
        sys.path.insert(0, _p)

from contextlib import ExitStack

import numpy as np

import concourse.bass as bass
import concourse.tile as tile
from concourse import bacc, mybir
from concourse.bass_utils import run_bass_kernel_spmd

try:
    import ml_dtypes

    BF16 = np.dtype(ml_dtypes.bfloat16)
except ImportError:  # pragma: no cover
    BF16 = None

N, D, K = 131072, 768, 2000
NCORES = 8
NSH = N // NCORES  # 16384 rows per core
P = 128
DT = D // P  # 6 contraction tiles
# score chunks, each within one PSUM bank (<=512 fp32)
KOFF = [0, 512, 1024, 1536]
KW = [512, 512, 512, 464]
NB = 4

F32 = mybir.dt.float32
BF = mybir.dt.bfloat16
U32 = mybir.dt.uint32


def build_nc_screen(n_rows: int = NSH):
    """Phase-1 screening program: single bf16 matmul pass.

    Bias (-0.5||c||^2) is folded into the matmul as two extra contraction
    rows (ones-weights x [bias_hi; bias_lo]) so the vector engine only runs
    max/max_index. Outputs the argmax index and the top-2 score values per
    row; rows with a small top-2 margin get recomputed exactly in phase 2.
    """
    assert n_rows % P == 0
    nt = n_rows // P
    nc = bacc.Bacc("TRN2", target_bir_lowering=False, debug=False)

    x = nc.dram_tensor("x", [n_rows, D], F32, kind="ExternalInput").ap()
    c_in = nc.dram_tensor("c", [D, K], BF, kind="ExternalInput").ap()
    bias2 = nc.dram_tensor("bias2", [2, K], BF, kind="ExternalInput").ap()
    ones = nc.dram_tensor("ones", [2, P], BF, kind="ExternalInput").ap()
    ident = nc.dram_tensor("ident", [P, P], BF, kind="ExternalInput").ap()
    out = nc.dram_tensor("out", [n_rows, 1], U32, kind="ExternalOutput").ap()
    vals = nc.dram_tensor("vals", [n_rows, 2], F32, kind="ExternalOutput").ap()

    with tile.TileContext(nc) as tc, ExitStack() as ctx:
        const = ctx.enter_context(tc.tile_pool(name="const", bufs=1))
        xin_p = ctx.enter_context(tc.tile_pool(name="xin", bufs=3))
        xcast_p = ctx.enter_context(tc.tile_pool(name="xcast", bufs=2))
        xtp_p = ctx.enter_context(tc.tile_pool(name="xtp", bufs=2, space="PSUM"))
        xts_p = ctx.enter_context(tc.tile_pool(name="xts", bufs=2))
        sc_p = ctx.enter_context(tc.tile_pool(name="sc", bufs=1, space="PSUM"))
        ss_p = ctx.enter_context(tc.tile_pool(name="ss", bufs=2))
        mx_p = ctx.enter_context(tc.tile_pool(name="mx", bufs=4))

        c3 = c_in.rearrange("(t p) k -> t p k", p=P)
        c_tiles = []
        for d in range(DT):
            ct = const.tile([P, K], BF, tag=f"c_{d}")
            nc.sync.dma_start(ct[:], c3[d])
            c_tiles.append(ct)
        bias_t = const.tile([2, K], BF, tag="bias2")
        nc.sync.dma_start(bias_t[:], bias2[:, :])
        ones_t = const.tile([2, P], BF, tag="ones")
        nc.sync.dma_start(ones_t[:], ones[:, :])
        id_t = const.tile([P, P], BF, tag="ident")
        nc.sync.dma_start(id_t[:], ident[:, :])

        for t in range(nt):
            xin = xin_p.tile([P, D], F32)
            nc.sync.dma_start(xin[:], x[t * P:(t + 1) * P, :])
            xh = xcast_p.tile([P, D], BF, tag="xh")
            nc.scalar.copy(xh[:], xin[:])
            xtp = xtp_p.tile([P, D], BF)
            for d in range(DT):
                nc.tensor.transpose(
                    xtp[:, d * P:(d + 1) * P], xh[:, d * P:(d + 1) * P], id_t[:])
            xts = xts_p.tile([P, D], BF)
            nc.scalar.copy(xts[:], xtp[:])

            banks = []
            for b in range(NB):
                # first two banks double-buffered: they are what the next
                # tile's first matmuls wait on (PSUM budget: 2+2+1+1 + 2 xtp)
                bank_tile = sc_p.tile([P, KW[b]], F32, tag=f"b{b}",
                                      name=f"bank{b}", bufs=(2 if b < 2 else 1))
                banks.append(bank_tile)
            # bias rows first so each bank's accumulation closes on d == DT-1
            for b in range(NB):
                nc.tensor.matmul(
                    banks[b][:], ones_t[:],
                    bias_t[:, KOFF[b]:KOFF[b] + KW[b]],
                    start=True, stop=False)
            for d in range(DT):
                for b in range(NB):
                    nc.tensor.matmul(
                        banks[b][:], xts[:, d * P:(d + 1) * P],
                        c_tiles[d][:, KOFF[b]:KOFF[b] + KW[b]],
                        start=False, stop=(d == DT - 1))

            ss = ss_p.tile([P, K], F32)
            for b in range(NB):
                nc.scalar.copy(ss[:, KOFF[b]:KOFF[b] + KW[b]], banks[b][:])

            mxv = mx_p.tile([P, 8], F32, tag="mxv")
            nc.vector.max(mxv[:], ss[:])
            mxi = mx_p.tile([P, 8], U32, tag="mxi")
            nc.vector.max_index(mxi[:], mxv[:], ss[:])
            nc.sync.dma_start(out[t * P:(t + 1) * P, :], mxi[:, 0:1])
            nc.sync.dma_start(vals[t * P:(t + 1) * P, :], mxv[:, 0:2])

    nc.compile()
    return nc


def build_nc(mode: str = "bf16x3", n_rows: int = NSH):
    """Build + compile the per-core Bass program.

    mode: 'bf16x3' (hi/lo split, 3 bf16 passes), 'fp32', 'fp32r', 'bf16'
    """
    assert n_rows % P == 0
    nt = n_rows // P
    nc = bacc.Bacc("TRN2", target_bir_lowering=False, debug=False)

    x = nc.dram_tensor("x", [n_rows, D], F32, kind="ExternalInput").ap()
    bias = nc.dram_tensor("bias", [P, K], F32, kind="ExternalInput").ap()
    out = nc.dram_tensor("out", [n_rows, 1], U32, kind="ExternalOutput").ap()

    split = mode == "bf16x3"
    cdt = BF if mode in ("bf16x3", "bf16") else F32
    mmdt = {"bf16x3": BF, "bf16": BF, "fp32": F32, "fp32r": mybir.dt.float32r}[mode]

    if split:
        c_hi = nc.dram_tensor("c_hi", [D, K], BF, kind="ExternalInput").ap()
        c_lo = nc.dram_tensor("c_lo", [D, K], BF, kind="ExternalInput").ap()
        c_srcs = [c_hi, c_lo]
    else:
        c_full = nc.dram_tensor("c", [D, K], cdt, kind="ExternalInput").ap()
        c_srcs = [c_full]
    ident = nc.dram_tensor("ident", [P, P], mmdt if mmdt != mybir.dt.float32r else F32,
                           kind="ExternalInput").ap()

    with tile.TileContext(nc) as tc, ExitStack() as ctx:
        const = ctx.enter_context(tc.tile_pool(name="const", bufs=1))
        xin_p = ctx.enter_context(tc.tile_pool(name="xin", bufs=3))
        xcast_p = ctx.enter_context(tc.tile_pool(name="xcast", bufs=2))
        xtp_p = ctx.enter_context(tc.tile_pool(name="xtp", bufs=2, space="PSUM"))
        xts_p = ctx.enter_context(tc.tile_pool(name="xts", bufs=2))
        sc_p = ctx.enter_context(tc.tile_pool(name="sc", bufs=1, space="PSUM"))
        ss_p = ctx.enter_context(tc.tile_pool(name="ss", bufs=2))
        mx_p = ctx.enter_context(tc.tile_pool(name="mx", bufs=4))

        # centroids resident in SBUF: [DT][P, K] per source (hi/lo or single)
        c_tiles = []
        for si, csrc in enumerate(c_srcs):
            c3 = csrc.rearrange("(t p) k -> t p k", p=P)
            tiles = []
            for d in range(DT):
                ct = const.tile([P, K], cdt, tag=f"c{si}_{d}")
                nc.sync.dma_start(ct[:], c3[d])
                tiles.append(ct)
            c_tiles.append(tiles)

        bias_t = const.tile([P, K], F32, tag="bias")
        nc.sync.dma_start(bias_t[:], bias[:, :])
        id_t = const.tile([P, P], ident.dtype, tag="ident")
        nc.sync.dma_start(id_t[:], ident[:, :])

        for t in range(nt):
            xin = xin_p.tile([P, D], F32)
            nc.sync.dma_start(xin[:], x[t * P:(t + 1) * P, :])

            if split:
                xh = xcast_p.tile([P, D], BF, tag="xh")
                nc.scalar.copy(xh[:], xin[:])
                xl = xcast_p.tile([P, D], BF, tag="xl")
                nc.vector.tensor_sub(xl[:], xin[:], xh[:])
                tsrc = [xh, xl]
            elif mode == "bf16":
                xh = xcast_p.tile([P, D], BF, tag="xh")
                nc.scalar.copy(xh[:], xin[:])
                tsrc = [xh]
            else:
                tsrc = [xin]

            # transpose x tiles -> [d, n] layout for matmul weights
            nsrc = len(tsrc)
            tdt = BF if cdt == BF else F32
            xtp = xtp_p.tile([P, D * nsrc], tdt)
            for si, xsrc in enumerate(tsrc):
                for d in range(DT):
                    nc.tensor.transpose(
                        xtp[:, si * D + d * P: si * D + (d + 1) * P],
                        xsrc[:, d * P:(d + 1) * P],
                        id_t[:],
                    )
            xts = xts_p.tile([P, D * nsrc], tdt)
            nc.scalar.copy(xts[:], xtp[:])

            def w(si, d):
                return xts[:, si * D + d * P: si * D + (d + 1) * P]

            banks = []
            for b in range(NB):
                bank_tile = sc_p.tile([P, KW[b]], F32, tag=f"b{b}", name=f"bank{b}")
                banks.append(bank_tile)
            if split:
                # accumulate xh.ch + xh.cl + xl.ch over d
                for d in range(DT):
                    for b in range(NB):
                        nc.tensor.matmul(
                            banks[b][:], w(0, d),
                            c_tiles[0][d][:, KOFF[b]:KOFF[b] + KW[b]],
                            start=(d == 0), stop=False)
                    for b in range(NB):
                        nc.tensor.matmul(
                            banks[b][:], w(0, d),
                            c_tiles[1][d][:, KOFF[b]:KOFF[b] + KW[b]],
                            start=False, stop=False)
                    for b in range(NB):
                        nc.tensor.matmul(
                            banks[b][:], w(1, d),
                            c_tiles[0][d][:, KOFF[b]:KOFF[b] + KW[b]],
                            start=False, stop=(d == DT - 1))
            else:
                for d in range(DT):
                    for b in range(NB):
                        lhs = w(0, d)
                        rhs = c_tiles[0][d][:, KOFF[b]:KOFF[b] + KW[b]]
                        if mode == "fp32r":
                            lhs = lhs.bitcast(mybir.dt.float32r)
                            rhs = rhs.bitcast(mybir.dt.float32r)
                        nc.tensor.matmul(banks[b][:], lhs, rhs,
                                         start=(d == 0), stop=(d == DT - 1))

            ss = ss_p.tile([P, K], F32)
            for b in range(NB):
                nc.vector.tensor_add(
                    ss[:, KOFF[b]:KOFF[b] + KW[b]], banks[b][:],
                    bias_t[:, KOFF[b]:KOFF[b] + KW[b]])

            mxv = mx_p.tile([P, 8], F32, tag="mxv")
            nc.vector.max(mxv[:], ss[:])
            mxi = mx_p.tile([P, 8], U32, tag="mxi")
            nc.vector.max_index(mxi[:], mxv[:], ss[:])
            nc.sync.dma_start(out[t * P:(t + 1) * P, :], mxi[:, 0:1])

    nc.compile()
    return nc


def make_in_maps(x: np.ndarray, centroids: np.ndarray, mode: str = "bf16x3",
                 n_rows: int = NSH, n_cores: int = NCORES):
    x = np.ascontiguousarray(x, dtype=np.float32)
    c = np.ascontiguousarray(centroids, dtype=np.float32)
    c_norm = (c.astype(np.float64) ** 2).sum(axis=0)
    bias = np.broadcast_to((-0.5 * c_norm).astype(np.float32), (P, K)).copy()

    base = {"bias": bias}
    if mode == "bf16x3":
        c_hi = c.astype(BF16)
        c_lo = (c - c_hi.astype(np.float32)).astype(BF16)
        base["c_hi"] = c_hi
        base["c_lo"] = c_lo
        base["ident"] = np.eye(P, dtype=BF16)
    elif mode == "bf16":
        base["c"] = c.astype(BF16)
        base["ident"] = np.eye(P, dtype=BF16)
    else:
        base["c"] = c
        base["ident"] = np.eye(P, dtype=np.float32)

    in_maps = []
    for i in range(n_cores):
        m = dict(base)
        m["x"] = x[i * n_rows:(i + 1) * n_rows]
        in_maps.append(m)
    return in_maps


_NC_CACHE = {}
LAST_RESULTS = []  # (label, BassKernelResults) of the most recent kernel() call


def _run_spmd(nc, in_maps, label):
    kw = {}
    if os.environ.get("KMEANS_TRACE"):
        kw["trace"] = True
        kw["tmpdir"] = os.environ.get("KMEANS_TRACE_DIR", "/tmp/km_trace") + "_" + label
        os.makedirs(kw["tmpdir"], exist_ok=True)
    res = run_bass_kernel_spmd(nc, in_maps, core_ids=list(range(NCORES)), **kw)
    LAST_RESULTS.append((label, res))
    return res

# Phase-2 capacity: rows per core recomputed exactly. Margin threshold:
# empirical max bf16 score error on randn data is ~0.2; flag anything under
# 4x that. ~5% of rows get flagged at this threshold.
P2_ROWS = 1024
MARGIN_TH = None  # set below after calibration constant


def _cached_nc(key, builder):
    if key not in _NC_CACHE:
        _NC_CACHE[key] = builder()
    return _NC_CACHE[key]


def make_screen_in_maps(x: np.ndarray, centroids: np.ndarray,
                        n_rows: int = NSH, n_cores: int = NCORES):
    x = np.ascontiguousarray(x, dtype=np.float32)
    c = np.ascontiguousarray(centroids, dtype=np.float32)
    c_norm = (c.astype(np.float64) ** 2).sum(axis=0)
    bias = (-0.5 * c_norm).astype(np.float32)
    bias_hi = bias.astype(BF16)
    bias_lo = (bias - bias_hi.astype(np.float32)).astype(BF16)
    base = {
        "c": c.astype(BF16),
        "bias2": np.stack([bias_hi, bias_lo]),
        "ones": np.ones((2, P), dtype=BF16),
        "ident": np.eye(P, dtype=BF16),
    }
    in_maps = []
    for i in range(n_cores):
        m = dict(base)
        m["x"] = x[i * n_rows:(i + 1) * n_rows]
        in_maps.append(m)
    return in_maps


def _run_exact(x_rows: np.ndarray, centroids: np.ndarray, n_rows: int):
    """Run the exact (bf16x3) program on x_rows padded to n_rows*NCORES."""
    nc = _cached_nc(("bf16x3", n_rows), lambda: build_nc("bf16x3", n_rows))
    total = n_rows * NCORES
    xp = np.zeros((total, D), dtype=np.float32)
    xp[: len(x_rows)] = x_rows
    in_maps = make_in_maps(xp, centroids, mode="bf16x3", n_rows=n_rows)
    res = _run_spmd(nc, in_maps, "phase2")
    out = np.concatenate(
        [res.results[i]["out"].reshape(n_rows) for i in range(NCORES)])
    return out[: len(x_rows)], res


def kernel(x: np.ndarray, centroids: np.ndarray) -> np.ndarray:
    mode = os.environ.get("KMEANS_MODE", "hybrid")
    LAST_RESULTS.clear()
    x = np.asarray(x)
    centroids = np.asarray(centroids)

    if mode != "hybrid":
        nc = _cached_nc((mode, NSH), lambda: build_nc(mode=mode))
        in_maps = make_in_maps(x, centroids, mode=mode)
        res = _run_spmd(nc, in_maps, mode)
        parts = [res.results[i]["out"].reshape(NSH) for i in range(NCORES)]
        return np.concatenate(parts).astype(np.int32)

    # phase 1: bf16 screen with top-2 margins
    nc1 = _cached_nc(("screen", NSH), lambda: build_nc_screen(NSH))
    in_maps = make_screen_in_maps(x, centroids)
    res1 = _run_spmd(nc1, in_maps, "phase1")
    idx = np.concatenate(
        [res1.results[i]["out"].reshape(NSH) for i in range(NCORES)]
    ).astype(np.int32)
    vals = np.concatenate(
        [res1.results[i]["vals"].reshape(NSH, 2) for i in range(NCORES)])

    margin = vals[:, 0] - vals[:, 1]
    th = float(os.environ.get("KMEANS_MARGIN_TH", "0.8"))
    flagged = np.flatnonzero(margin < th)

    # phase 2: exact recompute of flagged rows; pick the smallest padded
    # program that covers the count, chunking in the (unexpected) overflow case
    sizes = [512, 1024, 1536, 2048]
    per_core = min((s for s in sizes if s * NCORES >= len(flagged)),
                   default=sizes[-1])
    cap = per_core * NCORES
    for s in range(0, len(flagged), cap):
        rows = flagged[s:s + cap]
        exact_idx, _ = _run_exact(x[rows], centroids, per_core)
        idx[rows] = exact_idx
    return idx
